# revision 12
# baseline (speedup 1.0000x reference)
"""DecoderTreeRNN Trainium2 kernel (8 NeuronCores, single SPMD launch).

  - Fully collective-free: data-parallel over batch B for the WHOLE kernel
    (8 examples = 256 leaves per core). Each core projects only its own
    leaves against the full vocab, so no leaf AllGather, no cross-core
    skew sensitivity, and the host reassembles rows directly.
  - Tree expansion: flipped dataflow — the state is the PE-stationary
    operand (nodes on partitions), weights move as [Whh_l^T | Whh_r^T]
    packed fp8 DoubleRow (K=256/pass). All levels run s-major
    (slice-outer) with 3-way split accumulation groups: the next stage's
    bias matmuls (zero deps) plug the transpose/gate-latency gap, its
    (k2 0-1) halves run between the previous level's s0 and s1
    transposes, and (k2 2-3) after — the PE never idles across level
    boundaries, keeping the HAM clock un-throttled. Gates run on Scalar
    (sigmoid/tanh from PSUM) and DVE in bf16.
  - Projection: each core's 2 row tiles (256 local leaves) times the
    full 32768-padded vocab in 64 chunks of 512 cols. W_out^T fp8 (32MB)
    streams through a 4-deep rotating SBUF pool (2048 cols/tile) on the
    SP DMA ring — prefetch starts during the tree, steady-state refill
    (5.6us/tile) beats consumption (7.7us/tile). The first projection
    chunks use the same k2-half split to cover the last level's slice-1
    gate latency. Raw logits stream out as bf16 on the ACT ring (halves
    the output DMA; logits are O(1) so bf16 is ~1e-3 relative); the host
    adds b_out and log-softmax-normalizes during unshard.
"""

import sys

for _p in ("/opt/trn_rl_repo",):
    if _p not in sys.path:
        sys.path.append(_p)

import numpy as np
import ml_dtypes

import concourse.bass as bass
from concourse import bacc, tile, mybir
from concourse import bass_utils
from contextlib import ExitStack

BF16 = mybir.dt.bfloat16
F32 = mybir.dt.float32
FP8 = mybir.dt.float8e4
AF = mybir.ActivationFunctionType
ALU = mybir.AluOpType
BFNP = ml_dtypes.bfloat16
F8NP = mybir.dt.np(FP8)

N_CORES = 8
CW = 512            # vocab chunk width == one fp32 PSUM bank
WOC = 4             # vocab chunks per streamed weight tile (2048 cols)
WOB = 4             # streamed weight tiles in flight


def _build(B, H, V, DEPTH):
    KT = H // 128            # contraction tiles (8)
    KT2 = KT // 2            # DoubleRow k-pairs (4)
    Bl = B // N_CORES        # examples per core (8)
    L = 1 << DEPTH           # leaves per example (32)
    NLOC = Bl * L            # local leaf count (256)
    RT = NLOC // 128         # local row tiles (2)
    Vpad = ((V + N_CORES * 128 - 1) // (N_CORES * 128)) * N_CORES * 128
    NCH = Vpad // CW         # vocab chunks (64)
    GH = 3 * H               # 3072
    W2C = 2 * GH             # both sides' gate columns (6144)
    assert B % N_CORES == 0 and H % 128 == 0 and Vpad % (CW * WOC) == 0
    assert NLOC == 256 and NCH % 8 == 0

    nc = bacc.Bacc("TRN2", target_bir_lowering=False, debug=False,
                   num_devices=N_CORES, dynamic_dma_scratch_size=2048)

    NP0 = max(Bl, 16)        # level-0 stationary padded: DoubleRow LDWEIGHTS
                             # needs k-pair step % 16 == 0 (16B SBUF lines)

    # ---------------- DRAM I/O ----------------
    enc8_d = nc.dram_tensor("enc8", [128, KT, NP0], FP8, kind="ExternalInput")
    encN_d = nc.dram_tensor("encN", [Bl, H], BF16, kind="ExternalInput")
    w2_d = nc.dram_tensor("w2", [8, 128, 2, 2, 1536], FP8,
                          kind="ExternalInput")
    wbias_d = nc.dram_tensor("wbias", [1, W2C], BF16, kind="ExternalInput")
    bihn_d = nc.dram_tensor("bihn", [128, 2 * H], BF16, kind="ExternalInput")
    ident_d = nc.dram_tensor("ident", [128, 128], BF16, kind="ExternalInput")
    NWT0 = Vpad // (CW * WOC)
    wo_d = nc.dram_tensor("wo", [NWT0, 128, KT2, 2, WOC * CW], FP8,
                          kind="ExternalInput")
    out_d = nc.dram_tensor("out", [NLOC, Vpad], BF16, kind="ExternalOutput")

    with tile.TileContext(nc) as tc, ExitStack() as ctx:
        wpool = ctx.enter_context(tc.tile_pool(name="wpool", bufs=1))
        cpool = ctx.enter_context(tc.tile_pool(name="const", bufs=1))
        # shared 6-bank PSUM pool for gate groups AND projection chunks,
        # plus a dedicated 2-bank transpose pool (decoupled so transposes
        # never wait on an open split-group's bank — that would deadlock).
        psp = ctx.enter_context(tc.tile_pool(name="ps", bufs=6, space="PSUM"))
        ptp = ctx.enter_context(tc.tile_pool(name="pstp", bufs=2,
                                             space="PSUM"))
        stp = ctx.enter_context(tc.tile_pool(name="state", bufs=2))
        cap = ctx.enter_context(tc.tile_pool(name="carry", bufs=2))
        gp = ctx.enter_context(tc.tile_pool(name="gates", bufs=3))
        wopA = ctx.enter_context(tc.tile_pool(name="woA", bufs=WOB // 2))
        wopB = ctx.enter_context(tc.tile_pool(name="woB", bufs=WOB // 2))
        lgp = ctx.enter_context(tc.tile_pool(name="logits", bufs=2))

        # ---- latency-critical small inputs on the ACT ring (bias first:
        # it gates the very first PE instruction)
        wbias_sb = cpool.tile([1, W2C], BF16, tag="wbias")
        nc.scalar.dma_start(wbias_sb[:], wbias_d.ap())
        enc8_sb = cpool.tile([128, KT, NP0], FP8, tag="enc8")
        nc.scalar.dma_start(enc8_sb[:], enc8_d.ap())
        ident_sb = cpool.tile([128, 128], BF16, tag="ident")
        nc.scalar.dma_start(ident_sb[:], ident_d.ap())
        encN_sb = cpool.tile([Bl, H], BF16, tag="encN")
        nc.scalar.dma_start(encN_sb[:], encN_d.ap())
        bihn_sb = cpool.tile([128, 2 * H], BF16, tag="bihn")
        nc.scalar.dma_start(bihn_sb[:], bihn_d.ap())
        ones_sb = cpool.tile([1, 128], BF16, tag="ones")
        nc.vector.memset(ones_sb[:], 1.0)

        # ---- recurrent weights on the SP ring in exact consumption
        # order: (slice, k2-half, side) blocks — the first ~0.8MB block
        # unblocks level 0's first half-groups.
        w2_sb = wpool.tile([128, KT2, 2, W2C], FP8, tag="w2", name="w2")
        bi = 0
        for s in range(2):
            for kh in range(2):
                for si in range(2):
                    t3 = s * 2 + si
                    nc.sync.dma_start(
                        w2_sb[:, 2 * kh:2 * kh + 2, :,
                              1536 * t3:1536 * (t3 + 1)],
                        w2_d.ap()[bi])
                    bi += 1
        # ---- streamed projection weights: tiles alternate between the
        # two DGE rings (even on SP behind w2, odd on ACT) so supply is
        # 2x one ring's bandwidth. The 4-deep pool WAR-throttles loads to
        # the projection's pace. Odd-tile DMAs are EMITTED later (first
        # two after the tree's carry shifts, the rest interleaved into
        # the projection loop) — emitting them here would head-of-line
        # block the ACT queue behind their WAR waits.
        NWT = NCH // WOC
        wo_t = [None] * NWT
        for j in range(0, NWT, 2):
            wo_t[j] = wopA.tile([128, KT2, 2, WOC * CW], FP8, tag="woA",
                                name=f"wo{j}")
        for j in range(1, NWT, 2):
            wo_t[j] = wopB.tile([128, KT2, 2, WOC * CW], FP8, tag="woB",
                                name=f"wo{j}")

        def emit_wo_dma(j, eng):
            eng.dma_start(wo_t[j][:], wo_d.ap()[j])

        for j in range(0, NWT, 2):
            emit_wo_dma(j, nc.sync)

        # ================= emission helpers =================
        # One gate group = PSUM accumulation group [bias K=1, then 4 fp8
        # DoubleRow k2 matmuls], emitted in three pieces: bias (no deps —
        # plugs the PE gap while the previous level's s0 gates finish),
        # k2 0-1 (needs s0 transposes), k2 2-3 (needs s1 transposes).
        def emit_bias(ps, np_, si, s, g):
            c = ((s * 2 + si) * 3 + g) * 512
            nc.tensor.matmul(ps[0:np_, :], ones_sb[0:1, 0:np_],
                             wbias_sb[0:1, c:c + 512],
                             start=True, stop=False, skip_group_check=True)

        def emit_k2s(ps, cur8, np_, si, s, g, k2s):
            c = ((s * 2 + si) * 3 + g) * 512
            for k2 in k2s:
                nc.tensor.matmul(
                    ps[0:np_, :], cur8[:, 2 * k2:2 * k2 + 2, 0:np_],
                    w2_sb[:, k2, :, c:c + 512],
                    perf_mode=mybir.MatmulPerfMode.DoubleRow,
                    start=False, stop=(k2 == KT2 - 1), skip_group_check=True)

        def alloc_ps(lvl, si, s):
            return [psp.tile([128, CW], F32, tag="ps",
                             name=f"ps{lvl}_{si}_{s}_{g}") for g in range(3)]

        # per-(side, slice) gate math, 512 wide:
        # r=sig(ps0), z=sig(ps1), t=tanh(bihn + r*ps2), h' = t + z*(h-t)
        def emit_gates(si, s, n, ps, hN, dst, dst_c0):
            r_t = gp.tile([128, CW], BF16, tag="r")
            nc.scalar.activation(r_t[0:n, :], ps[0][0:n, :], AF.Sigmoid)
            z_t = gp.tile([128, CW], BF16, tag="z")
            nc.scalar.activation(z_t[0:n, :], ps[1][0:n, :], AF.Sigmoid)
            t1 = gp.tile([128, CW], BF16, tag="t1")
            nc.vector.tensor_tensor(t1[0:n, :], r_t[0:n, :],
                                    ps[2][0:n, :], op=ALU.mult)
            cb = si * H + s * 512
            t1b = gp.tile([128, CW], BF16, tag="t1b")
            nc.vector.tensor_tensor(t1b[0:n, :], t1[0:n, :],
                                    bihn_sb[0:n, cb:cb + 512], op=ALU.add)
            t_t = gp.tile([128, CW], BF16, tag="t")
            nc.scalar.activation(t_t[0:n, :], t1b[0:n, :], AF.Tanh)
            u = gp.tile([128, CW], BF16, tag="u")
            nc.vector.scalar_tensor_tensor(
                u[0:n, :], t_t[0:n, :], -1.0,
                hN[0:n, 512 * s:512 * (s + 1)],
                op0=ALU.mult, op1=ALU.add)   # u = h - t
            nc.vector.tensor_tensor(u[0:n, :], u[0:n, :],
                                    z_t[0:n, :], op=ALU.mult)
            nc.vector.tensor_tensor(dst[0:n, dst_c0:dst_c0 + 512],
                                    u[0:n, :], t_t[0:n, :], op=ALU.add)

        # PE transposes of dst k-tiles [k0, k1) into the fp8 next-level
        # stationary hT8n[:, k, col0:col0+n]. Four transposes pack into
        # one PSUM bank so the PE stream never stalls on a copy-out that
        # is queued behind gate ops on DVE/ACT.
        def emit_xposes(lvl, srcs, hT8n, n, k0, k1):
            for si, src, col0 in srcs:
                tp = None
                for k in range(k0, k1):
                    slot = 128 * ((k - k0) % 4)
                    if slot == 0:
                        tp = ptp.tile([128, CW], BF16, tag="tp",
                                      name=f"tp{lvl}_{si}_{k}")
                    nc.tensor.matmul(
                        tp[:, slot:slot + n],
                        src[0:n, 128 * k:128 * (k + 1)],
                        ident_sb[0:n, 0:n],
                        is_transpose=True, skip_group_check=True)
                    if k % 2 == 0:
                        nc.vector.tensor_copy(
                            hT8n[:, k, col0:col0 + n], tp[:, slot:slot + n])
                    else:
                        nc.scalar.activation(
                            hT8n[:, k, col0:col0 + n], tp[:, slot:slot + n],
                            AF.Copy)

        # ================= tree expansion =================
        with nc.named_scope("tree"):
            cur8 = enc8_sb          # [128, KT, n] fp8 stationary
            hN = encN_sb            # [n, H] bf16 carry
            n = Bl
            pend_xpose = None       # previous level's s1 transposes
            leaves = None
            for lvl in range(DEPTH):
                last = lvl == DEPTH - 1
                np_ = max(n, 16)
                hT8n = stp.tile([128, KT, 2 * n], FP8,
                                tag="lv" if last else "st",
                                name=f"hT8n{lvl}", bufs=1 if last else None)
                if not last:
                    hNn = cap.tile([2 * n, H], BF16, tag="hN",
                                   name=f"hNn{lvl}")
                    hl = None
                else:
                    hNn = None
                    hl = cap.tile([n, H], BF16, tag="hl", name=f"hl{lvl}")
                hr = cap.tile([n, H], BF16, tag="hr", name=f"hr{lvl}")

                early = [(0, 0), (1, 0)]        # slice-0 units (l, r)
                rest = [(0, 1), (1, 1)]         # slice-1 units

                ps_of = {}
                for si, s in early:
                    ps_of[(si, s)] = alloc_ps(lvl, si, s)
                    for g in range(3):
                        emit_bias(ps_of[(si, s)][g], np_, si, s, g)
                # previous level's s0 transposes ran just before the bias
                # matmuls above (emitted at the bottom of the previous
                # iteration); now the first contraction halves:
                for si, s in early:
                    for g in range(3):
                        emit_k2s(ps_of[(si, s)][g], cur8, np_, si, s, g,
                                 range(2))
                if pend_xpose is not None:
                    pend_xpose()            # prev level's s1 transposes
                    pend_xpose = None
                for si, s in early:
                    for g in range(3):
                        emit_k2s(ps_of[(si, s)][g], cur8, np_, si, s, g,
                                 range(2, KT2))
                    dst = (hNn if not last else hl) if si == 0 else hr
                    emit_gates(si, s, n, ps_of[(si, s)], hN, dst, 512 * s)
                for si, s in rest:
                    ps = alloc_ps(lvl, si, s)
                    for g in range(3):
                        emit_bias(ps[g], np_, si, s, g)
                        emit_k2s(ps[g], cur8, np_, si, s, g, range(KT2))
                    dst = (hNn if not last else hl) if si == 0 else hr
                    emit_gates(si, s, n, ps, hN, dst, 512 * s)

                # s0 transposes (s0 gates finished during slice-1 matmuls)
                if not last:
                    srcs = [(0, hNn, 0), (1, hr, n)]
                else:
                    srcs = [(0, hl, 0), (1, hr, n)]
                emit_xposes(lvl, srcs, hT8n, n, 0, KT2)
                if not last:
                    # side-r carry block: partition shift via DMA
                    nc.scalar.dma_start(hNn[n:2 * n, :], hr[0:n, :])

                def _pend(lvl=lvl, srcs=srcs, hT8n=hT8n, n=n):
                    emit_xposes(lvl, srcs, hT8n, n, KT2, KT)
                pend_xpose = _pend
                if last:
                    leaves = hT8n
                else:
                    hN = hNn
                    cur8 = hT8n
                    n *= 2

        # ================= projection =================
        # 2 row tiles (256 local leaves) x 64 vocab chunks, chunk-major.
        # The first PSPLIT chunks split their k2s around the last level's
        # s1 transposes, exactly like tree levels do.
        PSPLIT = 3
        with nc.named_scope("proj"):
            # odd weight tiles on the ACT ring, emitted only now — after
            # every tree carry-shift DMA is already queued there
            emit_wo_dma(1, nc.scalar)
            emit_wo_dma(3, nc.scalar)
            def proj_mms(pp, rt, i, k2s):
                wt = wo_t[i // WOC]
                c = (i % WOC) * CW
                for k2 in k2s:
                    nc.tensor.matmul(
                        pp[:], leaves[:, 2 * k2:2 * k2 + 2,
                                      128 * rt:128 * (rt + 1)],
                        wt[:, k2, :, c:c + 512],
                        perf_mode=mybir.MatmulPerfMode.DoubleRow,
                        start=(k2 == 0), stop=(k2 == KT2 - 1))

            lg_of = {}

            def proj_out(pp, rt, i):
                j = i // 8
                if (rt, j) not in lg_of:
                    lg_of[(rt, j)] = lgp.tile([128, 8 * CW], BF16,
                                              tag=f"lg{rt}", name=f"lg{rt}_{j}")
                lg = lg_of[(rt, j)]
                c = (i % 8) * CW
                if i % 2 == 0:
                    nc.vector.tensor_copy(lg[:, c:c + CW], pp[:])
                else:
                    nc.scalar.activation(lg[:, c:c + CW], pp[:], AF.Copy)
                if i % 8 == 7:
                    nc.scalar.dma_start(
                        out_d.ap()[128 * rt:128 * (rt + 1),
                                   CW * 8 * j:CW * 8 * (j + 1)], lg[:])

            pps = {}
            for i in range(PSPLIT):
                for rt in range(RT):
                    pps[(rt, i)] = psp.tile([128, CW], F32, tag="ps",
                                            name=f"pp{rt}_{i}")
                    proj_mms(pps[(rt, i)], rt, i, range(2))
            pend_xpose()                    # last level's s1 transposes
            for i in range(PSPLIT):
                for rt in range(RT):
                    proj_mms(pps[(rt, i)], rt, i, range(2, KT2))
                    proj_out(pps[(rt, i)], rt, i)
            for i in range(PSPLIT, NCH):
                # pace the remaining odd weight-tile DMAs into the ACT
                # queue three tiles ahead of consumption
                if i % WOC == 0:
                    ja = i // WOC + 3
                    if ja < NWT and ja % 2 == 1 and ja > 3:
                        emit_wo_dma(ja, nc.scalar)
                for rt in range(RT):
                    pp = psp.tile([128, CW], F32, tag="ps",
                                  name=f"pp{rt}_{i}")
                    proj_mms(pp, rt, i, range(KT2))
                    proj_out(pp, rt, i)

    nc.compile()
    return nc


_CACHE = {}


def _get(B, H, V, DEPTH):
    key = (B, H, V, DEPTH)
    if key not in _CACHE:
        _CACHE[key] = _build(B, H, V, DEPTH)
    return _CACHE[key]


def _leaf_perm(Bl, DEPTH):
    """Device leaf-column order -> (example e, reference leaf index t).

    Each level appends [left-children | right-children] in parent order,
    while the reference interleaves (new 2i = left(i), 2i+1 = right(i)).
    """
    cols = [(e, 0) for e in range(Bl)]
    for _ in range(DEPTH):
        cols = [(e, 2 * t) for (e, t) in cols] + \
               [(e, 2 * t + 1) for (e, t) in cols]
    return cols


def _pack_inputs(B, H, V, DEPTH, encoding, Whh_l, bih_l, bhh_l, Whh_r, bih_r,
                 bhh_r, W_out, b_out):
    """Host-side shard + transpose + cast. Returns in_maps for the 8 cores."""
    KT = H // 128
    KT2 = KT // 2
    Bl = B // N_CORES
    Vpad = ((V + N_CORES * 128 - 1) // (N_CORES * 128)) * N_CORES * 128
    GH = 3 * H

    # w2 moving operand, (slice, side)-major column order:
    # col' = ((s*2 + si)*3 + g)*512 + c  for gate g chunk (s, c) of side si
    w2cols = np.empty((H, 2 * GH), np.float32)
    wbias = np.empty((1, 2 * GH), np.float32)
    for si, (Whh, bih, bhh) in enumerate(
            ((Whh_l, bih_l, bhh_l), (Whh_r, bih_r, bhh_r))):
        WT = np.ascontiguousarray(Whh.T).astype(np.float32)  # [H, 3H]
        for s in range(2):
            for g in range(3):
                c0 = ((s * 2 + si) * 3 + g) * 512
                src = g * H + s * 512
                w2cols[:, c0:c0 + 512] = WT[:, src:src + 512]
                v = (bih + bhh) if g < 2 else bhh
                wbias[0, c0:c0 + 512] = v[src:src + 512]
    w2f = w2cols.reshape(KT2, 2, 128, 2 * GH).transpose(2, 0, 1, 3)
    w2 = np.empty((8, 128, 2, 2, 1536), np.float32)
    bi = 0
    for s in range(2):
        for kh in range(2):
            for si in range(2):
                t3 = s * 2 + si
                w2[bi] = w2f[:, 2 * kh:2 * kh + 2, :,
                             1536 * t3:1536 * (t3 + 1)]
                bi += 1
    w2 = np.ascontiguousarray(w2).astype(F8NP)
    wbias = wbias.astype(BFNP)

    # bih_n replicated over partitions: [128, 2H], col si*H + c
    bihn = np.empty((128, 2 * H), np.float32)
    bihn[:, 0:H] = np.asarray(bih_l)[2 * H:][None, :]
    bihn[:, H:2 * H] = np.asarray(bih_r)[2 * H:][None, :]
    bihn = np.ascontiguousarray(bihn).astype(BFNP)

    ident = np.eye(128, dtype=np.float32).astype(BFNP)

    # full projection weights, identical on every core (streamed on-chip)
    woT = np.zeros((H, Vpad), np.float32)
    woT[:, :V] = np.asarray(W_out).T
    NWT = Vpad // 2048
    wo = np.ascontiguousarray(
        woT.reshape(KT2, 2, 128, NWT, 2048).transpose(3, 2, 0, 1, 4)
    ).astype(F8NP)
    enc = np.asarray(encoding, np.float32)

    shared = {"w2": w2, "wbias": wbias, "bihn": bihn, "ident": ident,
              "wo": wo}
    in_maps = []
    for c in range(N_CORES):
        m = dict(shared)
        ec = enc[c * Bl:(c + 1) * Bl]                       # [Bl, H]
        m["encN"] = np.ascontiguousarray(ec).astype(BFNP)
        NP0 = max(Bl, 16)
        e8 = np.zeros((128, KT, NP0), np.float32)
        e8[:, :, :Bl] = ec.T.reshape(KT, 128, Bl).transpose(1, 0, 2)
        m["enc8"] = e8.astype(F8NP)
        in_maps.append(m)
    return in_maps


def _unshard(B, H, V, DEPTH, b_out, results):
    L = 1 << DEPTH
    Bl = B // N_CORES
    NLOC = Bl * L

    cols = _leaf_perm(Bl, DEPTH)                    # 256 entries
    e_of = np.array([e for e, t in cols])
    t_of = np.array([t for e, t in cols])

    full = np.empty((B, L, V), np.float32)
    bo = np.asarray(b_out, np.float32)[None, :]
    for c in range(N_CORES):
        o = results[c]["out"]                       # [NLOC, Vpad] bf16
        g = o[:, :V].astype(np.float32) + bo
        ex = np.exp(g, dtype=np.float64)
        lse = np.log(ex.sum(axis=1)).astype(np.float32)
        g -= lse[:, None]
        full[c * Bl + e_of, t_of] = g
    return full


def _run(B, H, V, DEPTH, inputs, trace=False, nc=None):
    if nc is None:
        nc = _get(B, H, V, DEPTH)
    in_maps = _pack_inputs(B, H, V, DEPTH, **{k: v for k, v in inputs.items()
                                              if k != "b_out"},
                           b_out=inputs["b_out"])
    res = bass_utils.run_bass_kernel_spmd(
        nc, in_maps, core_ids=list(range(N_CORES)), trace=trace)
    full = _unshard(B, H, V, DEPTH, inputs["b_out"], res.results)
    return full, res


def kernel(**inputs):
    enc = np.asarray(inputs["encoding"], np.float32)
    B, H = enc.shape
    V = np.asarray(inputs["W_out"]).shape[0]
    DEPTH = int(inputs["depth"])
    args = {k: np.asarray(v, np.float32) for k, v in inputs.items()
            if k != "depth"}
    full, _ = _run(B, H, V, DEPTH, args)
    return full
